# revision 20
# baseline (speedup 1.0000x reference)
"""Cross-attention Trainium2 kernel (v3: bf16 + fp8-DoubleRow scores).

Problem: B=8, SQ=SKV=2048, HIDDEN=256, fp32.
  Q = query @ Wq.T + bq ; K = key @ Wk.T + bk ; V = value @ Wv.T + bv
  out = softmax(Q @ K.T / sqrt(128)) @ V

Sharding: data-parallel over batch - one batch element per NeuronCore,
8 cores, no collectives.

Numerics (validated vs the reference on CPU + HW, harness gate 2e-2):
  inputs/weights bf16, projections bf16, Q/K quantized to fp8e4m3 for
  the scores matmul (fp8 DoubleRow, 256-deep contraction per pass),
  exp partly on ACT (Exp LUT) and partly on DVE (Schraudolph int16
  bit trick: bits of bf16(exp(x)) ~ x*2^7/ln2 + 16248.6+0.5), U and V
  in bf16 for the AV matmul.  Measured HW rel err ~1.2e-2.

Layout/scheduling notes:
  - whole K/V/Q tensors DMA'd up front, one instruction each, spread
    over the SP and Pool DMA queues (per-block loads cost ~1us of
    descriptor generation each and serialized the first 15us).
  - PE emission order per q-block: projection(qb), AV(qb-1),
    scores(qb).  AV covers the DVE latency of the qt8 eviction so the
    PE never stalls at the block boundary (HAM re-throttle).
  - V projection lands 4 k-chunks in one [128,1024] PSUM tile and is
    evicted by a single DVE tensor_tensor (+bv, wide bias tile) per
    group: 4 instructions instead of 16.
  - softmax denominator = ones-columns of Vb (col 256; col 257 pads
    the free dim even); finalize = DVE reciprocal + ACT copy-scale
    (DVE reading its own just-produced reciprocal crashes the device).
"""

import numpy as np

B, SQ, SKV, H = 8, 2048, 2048, 256
SCALE = float(np.sqrt(H / 2.0))
N_CORES = 8

P = 128          # partitions
DC = H // P      # d chunks (2)
EC = H // P      # e chunks (2)
NB = SQ // 512   # 512-row seq blocks (4)
KC = SKV // P    # k chunks (16)
NPAIR = KC // 2  # kc pairs per q block (8)
N_SCHR = 4       # pairs per block on DVE via Schraudolph (of 8)

# Schraudolph constants for bf16-bit output (int16 convert truncates;
# +0.5 recenters; -7.40 zero-means the piecewise-linear error)
SCHR_S = 128.0 / float(np.log(2.0))
SCHR_B = 127.0 * 128.0 - 7.40 + 0.5

_CACHE: dict = {}


def _emit(ctx, tc, aps):
    from concourse import mybir

    nc = tc.nc
    f32 = mybir.dt.float32
    bf16 = mybir.dt.bfloat16
    fp8 = mybir.dt.float8e4
    i16 = mybir.dt.int16
    AF = mybir.ActivationFunctionType
    DR = mybir.MatmulPerfMode.DoubleRow
    queryT, keyT, valueT, wqT, wkT, wvT, bq2, bk2, bv4 = aps[:9]
    out = aps[9]
    inv_scale = 1.0 / SCALE

    const_pool = ctx.enter_context(tc.tile_pool(name="const", bufs=1))
    ktv_pool = ctx.enter_context(tc.tile_pool(name="ktv", bufs=1))
    qt_pool = ctx.enter_context(tc.tile_pool(name="qt", bufs=4))
    u_pool = ctx.enter_context(tc.tile_pool(name="u", bufs=18))
    out_pool = ctx.enter_context(tc.tile_pool(name="outp", bufs=4))
    rec_pool = ctx.enter_context(tc.tile_pool(name="rec", bufs=4))
    # PSUM budget (8 banks of 2KB): pst 2x2 + proj 2x1 + pav 2x1 banks
    ps_s = ctx.enter_context(tc.tile_pool(name="ps_s", bufs=2, space="PSUM"))
    ps_p = ctx.enter_context(tc.tile_pool(name="ps_p", bufs=2, space="PSUM"))
    ps_av = ctx.enter_context(tc.tile_pool(name="ps_av", bufs=2, space="PSUM"))

    # ---- input DMAs: per-512-block, all on the one SP HWDGE queue ----
    # A single queue executes transfers IN ORDER, so each block gets the
    # full HBM bandwidth and the first K block lands ~1.5us in (three
    # whole-tensor DMAs would share bandwidth three ways and delay the
    # first matmul to ~14us).  Order: K blocks, Q block 0 (needed by the
    # early Q projection), V blocks, remaining Q blocks.
    def load_block(src, blk, tag):
        t = const_pool.tile([P, DC, 512], bf16, tag=f"{tag}{blk}")
        nc.sync.dma_start(
            t, src[:, blk * 512:(blk + 1) * 512].rearrange("(c p) s -> p c s", p=P)
        )
        return t

    def load_weight(name, src_ap):
        w = const_pool.tile([P, DC, H], bf16, tag=name)
        nc.sync.dma_start(w, src_ap.rearrange("(c p) e -> p c e", p=P))
        return w

    wk_sb = load_weight("wk", wkT)
    kin_t = [load_block(keyT, blk, "kin") for blk in range(NB)]
    bk_sb = const_pool.tile([P, EC], f32)
    nc.sync.dma_start(bk_sb, bk2.rearrange("c p -> p c"))
    wq_sb = load_weight("wq", wqT)
    bq_sb = const_pool.tile([P, EC], f32)
    nc.sync.dma_start(bq_sb, bq2.rearrange("c p -> p c"))
    qin_t = [None] * NB
    qin_t[0] = load_block(queryT, 0, "qin")
    wv_sb = load_weight("wv", wvT)
    bv_row = const_pool.tile([1, 4 * H], f32)
    nc.sync.dma_start(bv_row, bv4)
    vin_t = [load_block(valueT, blk, "vin") for blk in range(NB)]
    for blk in range(1, NB):
        qin_t[blk] = load_block(queryT, blk, "qin")
    bv_rep = const_pool.tile([P, 4 * H], f32)
    nc.gpsimd.partition_broadcast(bv_rep, bv_row)

    # ---- persistent per-core tensors ----
    KT8 = ktv_pool.tile([P, EC, SKV], fp8)      # [e_part, ec, k]
    Vb = ktv_pool.tile([P, KC, H + 2], bf16)    # [k_part, kc, e | one one]
    for kc in range(KC):
        nc.vector.tensor_scalar(
            Vb[:, kc, H:H + 2], bv_rep[:, 0:2], 0.0, 1.0,
            mybir.AluOpType.mult, mybir.AluOpType.add,
        )

    # ---- query blocks: pipelined proj -> AV(prev) -> scores -> exp ----
    def emit_proj(qb):
        qt8 = qt_pool.tile([P, EC, 512], fp8, tag="qt8")
        for ec in range(EC):
            pq = ps_p.tile([P, 512], f32, tag="ps_p")
            for dc in range(DC):
                nc.tensor.matmul(
                    pq,
                    lhsT=wq_sb[:, dc, ec * P:(ec + 1) * P],
                    rhs=qin_t[qb][:, dc, :],
                    start=(dc == 0),
                    stop=(dc == DC - 1),
                )
            nc.vector.tensor_scalar(
                qt8[:, ec, :], pq, bq_sb[:, ec:ec + 1], None, mybir.AluOpType.add,
            )
        return qt8

    def emit_scores_pair(qb, qt8, g, us):
        n_schr = N_SCHR
        pst = ps_s.tile([P, 1024], f32, tag="pst")
        for hh in range(2):
            kc = 2 * g + hh
            nc.tensor.matmul(
                pst[:, hh * 512:(hh + 1) * 512],
                lhsT=KT8[:, :, kc * P:(kc + 1) * P],
                rhs=qt8,
                start=True,
                stop=True,
                perf_mode=DR,
            )
        if g < NPAIR - n_schr:
            u = u_pool.tile([P, 1024], bf16, tag="u")
            nc.scalar.activation(u, pst, AF.Exp, scale=inv_scale)
        else:
            u16 = u_pool.tile([P, 1024], i16, tag="u16")
            nc.vector.tensor_scalar(
                u16, pst, SCHR_S * inv_scale, SCHR_B,
                mybir.AluOpType.mult, mybir.AluOpType.add,
            )
            u = u16.bitcast(bf16)
        us.append(u)

    def emit_av_chunk(qb, us, qs):
        pav = ps_av.tile([P, H + 2], f32, tag="pav")
        for g in range(NPAIR):
            u = us[g]
            for hh in range(2):
                kc = 2 * g + hh
                nc.tensor.matmul(
                    pav,
                    lhsT=u[:, hh * 512 + qs * P: hh * 512 + (qs + 1) * P],
                    rhs=Vb[:, kc, :],
                    start=(kc == 0),
                    stop=(kc == KC - 1),
                )
        rec = rec_pool.tile([P, 1], f32, tag="rec")
        nc.vector.reciprocal(rec, pav[:, H:H + 1])
        ot = out_pool.tile([P, H], f32, tag="ot")
        nc.scalar.activation(ot, pav[:, 0:H], AF.Copy, scale=rec)
        nc.sync.dma_start(
            out[qb * 512 + qs * P: qb * 512 + (qs + 1) * P, :], ot
        )

    # ---- key: project into KT8 (fp8, bias fused) ----
    for blk in range(NB):
        cols = slice(blk * 512, (blk + 1) * 512)
        for ec in range(EC):
            pk = ps_p.tile([P, 512], f32, tag="ps_p")
            for dc in range(DC):
                nc.tensor.matmul(
                    pk,
                    lhsT=wk_sb[:, dc, ec * P:(ec + 1) * P],
                    rhs=kin_t[blk][:, dc, :],
                    start=(dc == 0),
                    stop=(dc == DC - 1),
                )
            nc.vector.tensor_scalar(
                KT8[:, ec, cols],
                pk, bk_sb[:, ec:ec + 1], None, mybir.AluOpType.add,
            )

    # ---- Q projection of block 0, early: its DVE eviction drains
    # while the PE runs the V projection, so scores(0) start stall-free
    qt8_next = emit_proj(0)

    # ---- value: project into Vb (+bv), 4 k-chunks per PSUM tile ----
    for blk in range(NB):
        pv4 = ps_s.tile([P, 1024], f32, tag="pst")
        for j in range(4):
            kc = blk * 4 + j
            for dc in range(DC):
                nc.tensor.matmul(
                    pv4[:, j * H:(j + 1) * H],
                    lhsT=vin_t[blk][:, dc, j * P:(j + 1) * P],
                    rhs=wv_sb[:, dc, :],
                    start=(dc == 0),
                    stop=(dc == DC - 1),
                )
        nc.vector.tensor_add(
            Vb[:, blk * 4:(blk + 1) * 4, 0:H], pv4, bv_rep,
        )

    # Pipeline: qt8(qb) is evicted one iteration ahead so scores(qb)
    # start with no DVE wait.  AV chunks of qb-1 are INTERLEAVED between
    # scores pairs: the in-order PE queue then always has ready matmuls
    # while the pst ring (2 bufs) throttles scores to ACT/DVE exp pace.
    pending = None  # (qb, us) awaiting AV+finalize
    for qb in range(NB):
        qt8 = qt8_next
        us = []
        for g in range(NPAIR):
            emit_scores_pair(qb, qt8, g, us)
            if g % 2 == 1 and pending is not None:
                emit_av_chunk(pending[0], pending[1], g // 2)
        if qb + 1 < NB:
            qt8_next = emit_proj(qb + 1)
        pending = (qb, us)
    for qs in range(4):
        emit_av_chunk(pending[0], pending[1], qs)


def _build():
    from contextlib import ExitStack

    import concourse.tile as tile
    from concourse import bacc, mybir

    f32 = mybir.dt.float32
    bf16 = mybir.dt.bfloat16
    nc = bacc.Bacc(
        "TRN2", target_bir_lowering=False, debug=False, num_devices=N_CORES
    )
    queryT = nc.dram_tensor("queryT", [H, SQ], bf16, kind="ExternalInput").ap()
    keyT = nc.dram_tensor("keyT", [H, SKV], bf16, kind="ExternalInput").ap()
    valueT = nc.dram_tensor("valueT", [H, SKV], bf16, kind="ExternalInput").ap()
    wqT = nc.dram_tensor("wqT", [H, H], bf16, kind="ExternalInput").ap()
    wkT = nc.dram_tensor("wkT", [H, H], bf16, kind="ExternalInput").ap()
    wvT = nc.dram_tensor("wvT", [H, H], bf16, kind="ExternalInput").ap()
    bq2 = nc.dram_tensor("bq2", [EC, P], f32, kind="ExternalInput").ap()
    bk2 = nc.dram_tensor("bk2", [EC, P], f32, kind="ExternalInput").ap()
    bv4 = nc.dram_tensor("bv4", [1, 4 * H], f32, kind="ExternalInput").ap()
    out = nc.dram_tensor("out", [SQ, H], f32, kind="ExternalOutput").ap()

    aps = (queryT, keyT, valueT, wqT, wkT, wvT, bq2, bk2, bv4, out)
    with tile.TileContext(nc) as tc, ExitStack() as ctx:
        _emit(ctx, tc, aps)
    nc.compile()
    return nc


def _get_nc():
    if "nc" not in _CACHE:
        _CACHE["nc"] = _build()
    return _CACHE["nc"]


def _in_maps(query, key, value, Wq, bq, Wk, bk, Wv, bv):
    import ml_dtypes

    bf = ml_dtypes.bfloat16
    q = np.asarray(query, np.float32)
    k = np.asarray(key, np.float32)
    v = np.asarray(value, np.float32)
    # [B, s, d] -> [B, d, s] bf16 layout prep, done host-side as part of
    # sharding (contraction dim on partitions; bf16 halves the DMA bytes).
    qT = np.ascontiguousarray(q.transpose(0, 2, 1)).astype(bf)
    kT = np.ascontiguousarray(k.transpose(0, 2, 1)).astype(bf)
    vT = np.ascontiguousarray(v.transpose(0, 2, 1)).astype(bf)
    wqT = np.ascontiguousarray(np.asarray(Wq, np.float32).T).astype(bf)
    wkT = np.ascontiguousarray(np.asarray(Wk, np.float32).T).astype(bf)
    wvT = np.ascontiguousarray(np.asarray(Wv, np.float32).T).astype(bf)
    bq2 = np.ascontiguousarray(np.asarray(bq, np.float32).reshape(EC, P))
    bk2 = np.ascontiguousarray(np.asarray(bk, np.float32).reshape(EC, P))
    bv4 = np.ascontiguousarray(
        np.tile(np.asarray(bv, np.float32).reshape(1, H), (1, 4))
    )
    maps = []
    for b in range(B):
        maps.append(
            {
                "queryT": qT[b],
                "keyT": kT[b],
                "valueT": vT[b],
                "wqT": wqT,
                "wkT": wkT,
                "wvT": wvT,
                "bq2": bq2,
                "bk2": bk2,
                "bv4": bv4,
            }
        )
    return maps


def _run(in_maps, trace=False, **kw):
    import concourse.bass_utils as bass_utils

    if trace:
        # zero-egress container: skip the artifact upload step
        bass_utils.upload_artifacts = lambda tmpdir: f"local://{tmpdir}"
    nc = _get_nc()
    return bass_utils.run_bass_kernel_spmd(
        nc, in_maps, list(range(N_CORES)), trace=trace, **kw
    )


def kernel(query, key, value, Wq, bq, Wk, bk, Wv, bv):
    in_maps = _in_maps(query, key, value, Wq, bq, Wk, bk, Wv, bv)
    _run(in_maps)  # warmup execution (cold-start insurance)
    res = _run(in_maps)
    return np.stack([res.results[b]["out"] for b in range(B)], axis=0)
